# revision 52
# baseline (speedup 1.0000x reference)
"""Trainium2 Bass kernel for the AR(t) recurrence problem.

Math: the recurrence  x_i = sum_j params[j] * x_{i-1-j} + bias  (i in [t, 2t))
is affine in the seed window:  out = inputs @ M + c,  with M, c functions of
params/bias only. M factors exactly as M = T2 @ (I + T1h) where T2 is the
lower-triangular Toeplitz of p_rev (= params reversed) and T1h the upper-
triangular Toeplitz of the AR impulse-response tail h~. The alternating-sign
params make T1h's error-carrying content coherent/smooth, so T1h is
numerically low-rank: a rank-128 SVD (host, randomized) captures it to
~4e-4 output error. Device compute per core (512 batch rows, fp8e4,
DoubleRow = 2 contraction k-tiles per PE pass):

  stage-1 (72 mm): psum1[ci] = f.T tiles       f.T = T2^T s.T  (triangular)
  stage-A ( 8 mm): psumY = (sA*T2@U)^T s.T  -> Y8               (rank-128)
  stage-B (16 mm): psum2[it] = (sB*V)^T Y8 + 2*f8[it]           (DoubleRow
                   pairs the V-tile with an exact 2*I fp8 tile, adding the
                   identity term in the same pass)
  psum2 = 2^11 (f + corr).T -> fp8e4 (Act/DVE alternating) -> DMA
  host: /2^11 + c

Schedule notes (from perfetto-trace iteration; measured ~40.1-41.8us vs
the 42.4-43.2us baseline, run-to-run chip power state adds ~15% noise):
  - Only sync+scalar have HWDGE queues (gpsimd dma_start is slow SWDGE).
  - HWDGE early-phase throughput is ~(50-130 rows/us) x row-bytes per
    queue, so input chunks use 1-3KB rows (host permutes DRAM pair order
    to [7,5,3,1 | 6,4,2,0]) and each queue's entries are ordered by when
    the warm MM stream consumes them; the first usable pair lands ~2.8us
    after the first trigger no matter what.
  - Warm-up matmuls on a zeroed tile keep the PE gaplessly busy from
    ~2.5us into the measured window; the HAM clock gate (1.2 -> 2.4 GHz)
    flips ~3.7us after the last PE idle gap, so the real stream runs at
    warm 216ns/pass almost from its start.
  - stage-A is interleaved right after the stage-1 groups that consume
    the same input pairs; stage-B pairs spread through the back half.
  - Output is fp8e4 at 2^11 scale (data term absmax ~0.115 -> 236 < 448
    saturation), halving output DMA bytes; out DMAs alternate sync and
    scalar queues and the final pair is cast in engine-split halves and
    sent as two single-tile DMAs to shorten the tail.

fp8 is ample precision: the data term has magnitude ~0.0155 rms while
|out| ~ 1.8 (dominated by c, exact). Measured max elementwise rel error
~5.2e-3 (tolerance 2e-2).
"""

import numpy as np
import ml_dtypes

B = 4096          # batch rows
T = 2048          # time steps == contraction length
NCORES = 8
BS = B // NCORES  # 512 rows per core
P = 128           # partitions
NJ = T // P       # 16 contraction tiles
NPAIR = NJ // 2   # 8 DoubleRow contraction pairs
RANK = 128        # low-rank correction rank
W_SCALE = 1024.0  # 2^10 on T2 tiles
SA = 128.0        # 2^7 on Pt (stage-A)
SB = 16.0         # 2^4 on V (stage-B); psum2 scale = W_SCALE * 2 = SA*SB = 2^11
OUT_SCALE = 2048.0
NWARM = 7

E4 = ml_dtypes.float8_e4m3

_cache = {}


def _build_and_compile():
    import concourse.mybir as mybir
    from concourse import bacc
    from concourse.tile import TileContext

    nc = bacc.Bacc(
        "TRN2",
        target_bir_lowering=False,
        debug=False,
        enable_asserts=False,
        num_devices=NCORES,
    )
    in8 = nc.dram_tensor("art_in8", [P, NJ, BS], mybir.dt.float8e4, kind="ExternalInput")
    w8 = nc.dram_tensor("art_w8", [P, NJ + 1, P], mybir.dt.float8e4, kind="ExternalInput")
    p8 = nc.dram_tensor("art_p8", [P, NJ, RANK], mybir.dt.float8e4, kind="ExternalInput")
    vi8 = nc.dram_tensor("art_vi8", [P, NJ + 10, P], mybir.dt.float8e4, kind="ExternalInput")
    # out.T in [partition, k-tile-pair-slot, b] layout so 2-tile SBUF groups DMA
    # with matching AP shapes; host reassembles.
    outT = nc.dram_tensor("art_outT", [P, NJ, BS], mybir.dt.float8e4, kind="ExternalOutput")

    DR = mybir.MatmulPerfMode.DoubleRow

    with TileContext(nc) as tc:
        with (
            tc.tile_pool(name="wstk", bufs=1) as wpool,
            tc.tile_pool(name="pstk", bufs=1) as ppool,
            tc.tile_pool(name="vstk", bufs=1) as vpool,
            tc.tile_pool(name="warm", bufs=1) as wupool,
            tc.tile_pool(name="acts", bufs=NPAIR) as ipool,
            tc.tile_pool(name="yf", bufs=1) as yfpool,
            tc.tile_pool(name="outs", bufs=8) as opool,
            tc.tile_pool(name="ps1", bufs=4, space="PSUM") as f1pool,
            tc.tile_pool(name="ps2", bufs=4, space="PSUM") as c2pool,
        ):
            # W stack split into two tiles (overlapping at slot 4) so the
            # first stage-1 matmuls gate on an 80KB DMA, not the full stack.
            wta = wpool.tile([P, 9, P], mybir.dt.float8e4, name="wa_sb")
            wtb = wpool.tile([P, 9, P], mybir.dt.float8e4, name="wb_sb")
            pt_ = ppool.tile([P, NJ, RANK], mybir.dt.float8e4, name="p_sb")
            vt_ = vpool.tile([P, NJ + 10, P], mybir.dt.float8e4, name="v_sb")
            # three Y/f stacks (Y8 duplicated) so stage-B waves gate only on
            # their own quarter's f-casts and can run mid-stage-1.
            yfA = yfpool.tile([P, 9, BS], mybir.dt.float8e4, name="yfA_sb")
            yfD = yfpool.tile([P, 2, BS], mybir.dt.float8e4, name="yfD_sb")
            wu = wupool.tile([P, 2, BS], mybir.dt.float8e4, name="wu_sb")
            nc.vector.memset(wu[:], 0.0)
            nc.vector.memset(yfD[:, 1, :], 0.0)

            # input tiles; DRAM pair order (host korder) is
            # [7, 5, 3, 1, 6, 4, 2, 0] so each queue's chunks are contiguous
            # with 1-3KB rows (HWDGE throughput ~ rows/us x row-bytes).
            t7 = ipool.tile([P, 2, BS], mybir.dt.float8e4, tag="in", name="in_t7")
            t53 = ipool.tile([P, 4, BS], mybir.dt.float8e4, tag="in", name="in_t53")
            t1 = ipool.tile([P, 2, BS], mybir.dt.float8e4, tag="in", name="in_t1")
            t6 = ipool.tile([P, 2, BS], mybir.dt.float8e4, tag="in", name="in_t6")
            t420 = ipool.tile([P, 6, BS], mybir.dt.float8e4, tag="in", name="in_t420")
            rhs_map = {
                7: (t7, 0), 5: (t53, 0), 3: (t53, 2), 1: (t1, 0),
                6: (t6, 0), 4: (t420, 0), 2: (t420, 2), 0: (t420, 4),
            }

            def in_rhs(r):
                tl, s = rhs_map[r]
                return tl[:, s : s + 2, :]

            # ladder ordered by when the warm MM stream consumes each chunk
            # (HWDGE drains each queue's entries in order at a limited rate);
            # pt is split so stage-A(7..4) gates on a 128KB half.
            nc.sync.dma_start(out=wta[:], in_=w8[:, 0:9, :])
            nc.scalar.dma_start(out=t6[:], in_=in8[:, 8:10, :])
            nc.sync.dma_start(out=t7[:], in_=in8[:, 0:2, :])
            nc.scalar.dma_start(out=pt_[:, 8:16, :], in_=p8[:, 8:16, :])
            nc.sync.dma_start(out=t53[:], in_=in8[:, 2:6, :])
            nc.scalar.dma_start(out=t420[:], in_=in8[:, 10:16, :])
            nc.sync.dma_start(out=wtb[:], in_=w8[:, 8:17, :])
            nc.scalar.dma_start(out=pt_[:, 0:8, :], in_=p8[:, 0:8, :])
            nc.sync.dma_start(out=t1[:], in_=in8[:, 6:8, :])
            nc.sync.dma_start(out=vt_[:], in_=vi8[:])

            # warm-up matmuls keep the PE gaplessly busy from early on so
            # the HAM clock (1.2 -> 2.4 GHz, flips ~3.7us after the last PE
            # idle gap) is warm soon after the real stream starts.
            wps = c2pool.tile([P, BS], mybir.dt.float32, tag="c", name="warmps")
            for _ in range(NWARM):
                nc.tensor.matmul(wps[:], wu[:, :, :P], wu[:], perf_mode=DR)

            psY = c2pool.tile([P, BS], mybir.dt.float32, tag="c", name="psY")

            def stage_a(r):
                nc.tensor.matmul(
                    psY[:],
                    pt_[:, 2 * r : 2 * r + 2, :],
                    in_rhs(r),
                    start=(r == NPAIR - 1),
                    stop=(r == 0),
                    perf_mode=DR,
                )
                if r == 0:
                    nc.vector.tensor_scalar_mul(yfA[:, 0, :], psY[:], 1.0)
                    nc.scalar.copy(yfD[:, 0, :], psY[:])

            def stage_1(ci, r):
                ps1 = f1pool.tile([P, BS], mybir.dt.float32, tag="f", name=f"f{ci}")
                inbank = ci <= 7
                for rp in range(r, NPAIR):
                    q = 2 * rp - ci + 1
                    lhsT = wta[:, q : q + 2, :] if q <= 7 else wtb[:, q - 8 : q - 6, :]
                    nc.tensor.matmul(
                        ps1[:],
                        lhsT,
                        in_rhs(rp),
                        start=(rp == r),
                        stop=(rp == NPAIR - 1) and not inbank,
                        perf_mode=DR,
                    )
                if inbank:
                    # fold the rank correction into the f psum (extra DR pass
                    # pairing the 2^3-scaled V tile with a zero slot), then
                    # cast psum -> fp8 out directly: no f8 round-trip, so the
                    # tail does not wait on an f-cast before stage-B.
                    nc.tensor.matmul(
                        ps1[:],
                        vt_[:, NJ + 8 - ci : NJ + 10 - ci, :],
                        yfD[:, 0:2, :],
                        start=False,
                        stop=True,
                        perf_mode=DR,
                    )
                    ot = opool.tile([P, 1, BS], mybir.dt.float8e4, tag="o", name=f"od{ci}")
                    h = BS // 2
                    nc.vector.tensor_scalar_mul(ot[:, 0, :h], ps1[:, :h], 1.0)
                    nc.scalar.copy(ot[:, 0, h:], ps1[:, h:])
                    q = nc.scalar if ci % 2 == 1 else nc.sync
                    q.dma_start(out=outT[:, ci : ci + 1, :], in_=ot[:])
                    return
                # evacuate f.T tile to fp8, alternating engines (ci >= 8)
                dst = yfA[:, ci - 7, :]
                if ci % 2 == 0:
                    nc.vector.tensor_scalar_mul(dst, ps1[:], 1.0)
                else:
                    nc.scalar.copy(dst, ps1[:])

            oq = {"i": 0}

            def stage_b_pair(hi, last=False):
                lo = hi - 1
                ot = opool.tile([P, 2, BS], mybir.dt.float8e4, tag="o", name=f"o{lo}")
                for it, slot in ((hi, 1), (lo, 0)):
                    ps2 = c2pool.tile([P, BS], mybir.dt.float32, tag="c", name=f"c{it}")
                    rhs = yfA[:, 0 : it - 6 : it - 7, :]
                    nc.tensor.matmul(
                        ps2[:],
                        vt_[:, it : 17 : 16 - it, :],
                        rhs,
                        start=True,
                        stop=True,
                        perf_mode=DR,
                    )
                    if last:
                        # split the final casts across both engines and DMA
                        # each tile separately on its own queue
                        h = BS // 2
                        nc.vector.tensor_scalar_mul(ot[:, slot, :h], ps2[:, :h], 1.0)
                        nc.scalar.copy(ot[:, slot, h:], ps2[:, h:])
                        q = nc.scalar if slot == 1 else nc.sync
                        q.dma_start(out=outT[:, lo + slot : lo + slot + 1, :],
                                    in_=ot[:, slot : slot + 1, :])
                    elif it % 2 == 1:
                        nc.scalar.copy(ot[:, slot, :], ps2[:])
                    else:
                        nc.vector.tensor_scalar_mul(ot[:, slot, :], ps2[:], 1.0)
                if not last:
                    q = nc.sync if oq["i"] % 2 == 0 else nc.scalar
                    oq["i"] += 1
                    q.dma_start(out=outT[:, lo : lo + 2, :], in_=ot[:])

            # interleave: first stage-1 tiles gate on the earliest DMAs, then
            # all stage-A passes (Y8 ready early), stage-1 descending, with
            # stage-B tiles 8-15 inserted as soon as yfA is complete.
            stage_1(15, 7)
            stage_1(14, 7)
            stage_1(13, 6)
            stage_1(12, 6)
            stage_a(7)
            stage_a(6)
            stage_1(11, 5)
            stage_1(10, 5)
            stage_a(5)
            stage_1(9, 4)
            stage_a(4)
            stage_1(8, 4)
            stage_a(3)
            stage_a(2)
            stage_a(1)
            stage_a(0)
            stage_1(7, 3)
            stage_1(6, 3)
            for hi in (15, 13, 11, 9):
                stage_b_pair(hi)
            stage_1(5, 2)
            stage_1(4, 2)
            stage_1(3, 1)
            stage_1(2, 1)
            stage_1(1, 0)
            stage_1(0, 0)

    nc.compile()
    return nc


def _host_factors(params, bias):
    """All device operand tensors + c, from params/bias (float64 host math)."""
    t = T
    p_rev = params[::-1].astype(np.float64)

    # c: bias propagation through the recurrence
    b = np.float64(bias[0])
    u = np.zeros(t, np.float64)
    c = np.empty(t, np.float64)
    for k in range(t):
        nv = u @ p_rev + b
        c[k] = nv
        u = np.roll(u, -1)
        u[-1] = nv

    # h~: AR impulse response tail (h_0 = 1 excluded)
    a_full = np.concatenate([[0.0], params.astype(np.float64)])
    h = np.zeros(t)
    h[0] = 1.0
    for dd in range(1, t):
        h[dd] = a_full[1:dd + 1] @ h[dd - 1::-1][:dd]
    ht = h.copy()
    ht[0] = 0.0

    idx = np.arange(t)
    D = idx[:, None] - idx[None, :]
    T2 = np.where(D >= 0, p_rev[np.clip(D, 0, t - 1)], 0.0)
    T1h = np.where(-D >= 1, ht[np.clip(-D, 0, t - 1)], 0.0)

    # randomized SVD of the correction operator
    rng = np.random.default_rng(0)
    Q, _ = np.linalg.qr(T1h @ rng.standard_normal((t, RANK + 32)))
    u2, sig, vt = np.linalg.svd(Q.T @ T1h, full_matrices=False)
    U = (Q @ u2[:, :RANK]) * sig[:RANK]
    Vt = vt[:RANK]
    Pt = T2 @ U  # [t, RANK]

    # W stack: slot s=0..16 <-> tile-diagonal d=s-1; W[s][jw,kw]=1024*p_rev[128(s-1)+jw-kw]
    sidx = (128 * (np.arange(NJ + 1) - 1))[:, None, None] + idx[:P, None] - idx[None, :P]
    wvals = np.where(
        (sidx >= 0) & (sidx < t), (W_SCALE * p_rev)[np.clip(sidx, 0, t - 1)], 0.0
    )  # [17, 128, 128]
    w8 = np.ascontiguousarray(wvals.transpose(1, 0, 2).astype(np.float32)).astype(E4)

    p8 = np.ascontiguousarray(
        (SA * Pt).reshape(NJ, P, RANK).transpose(1, 0, 2).astype(np.float32)
    ).astype(E4)

    vi = np.empty((P, NJ + 10, P), np.float32)
    vi[:, :NJ, :] = (SB * Vt).reshape(P, NJ, P)
    vi[:, NJ, :] = 2.0 * np.eye(P, dtype=np.float32)
    # slots 17/18: V tiles 1 and 0 at W_SCALE/SA = 2^3, for the in-bank
    # correction passes of output tiles 1 and 0 (psum stays at 2^10);
    # slot 19: zeros (pairs with slot 18 in the tile-0 pass).
    vb = (8.0 * Vt).reshape(P, NJ, P)
    for k in range(8):
        vi[:, NJ + 1 + k, :] = vb[:, 7 - k, :]
    vi[:, NJ + 9, :] = 0.0
    vi8 = np.ascontiguousarray(vi).astype(E4)

    return w8, p8, vi8, c


def _make_in_maps(inputs, params, bias):
    key = (params.tobytes(), bias.tobytes())
    if _cache.get("fkey") == key:
        w8, p8, vi8, c = _cache["factors"]
    else:
        w8, p8, vi8, c = _host_factors(params, bias)
        _cache["fkey"] = key
        _cache["factors"] = (w8, p8, vi8, c)
    in8_full = inputs.astype(E4)
    # DRAM pair order expected by the kernel's DMA ladder:
    # [7, 5, 3, 1, 6, 4, 2, 0]  (pair r = k-tiles 2r, 2r+1)
    korder = [14, 15, 10, 11, 6, 7, 2, 3, 12, 13, 8, 9, 4, 5, 0, 1]
    in_maps = []
    for s in range(NCORES):
        shard = in8_full[s * BS : (s + 1) * BS, :]  # [BS, T]
        in8 = np.ascontiguousarray(
            shard.T.reshape(NJ, P, BS)[korder].transpose(1, 0, 2)
        )
        in_maps.append({"art_in8": in8, "art_w8": w8, "art_p8": p8, "art_vi8": vi8})
    return in_maps, c


def run(inputs, params, bias, **spmd_kwargs):
    """Build in_maps, run the SPMD kernel, return (output, BassKernelResults)."""
    from concourse.bass_utils import run_bass_kernel_spmd

    if "nc" not in _cache:
        _cache["nc"] = _build_and_compile()
    nc = _cache["nc"]

    inputs = np.ascontiguousarray(np.asarray(inputs, dtype=np.float32))
    params = np.asarray(params, dtype=np.float32)
    bias = np.asarray(bias, dtype=np.float32)
    assert inputs.shape == (B, T), inputs.shape
    assert params.shape == (T,), params.shape
    in_maps, c = _make_in_maps(inputs, params, bias)
    res = run_bass_kernel_spmd(nc, in_maps, core_ids=list(range(NCORES)), **spmd_kwargs)
    slot_scale = np.full(NJ, 1.0 / OUT_SCALE, np.float32)
    slot_scale[:8] = 1.0 / W_SCALE  # in-bank tiles (0-7) at 2^10
    row_scale = np.repeat(slot_scale, P)  # out.T row = 128*slot + p
    c32 = c.astype(np.float32)
    outs = []
    for r in res.results:
        # art_outT [128, 16, 512]: [p, slot, b] with out.T row = 128*slot + p
        oT = r["art_outT"].astype(np.float32).transpose(1, 0, 2).reshape(T, BS)
        outs.append(oT.T * row_scale[None, :] + c32[None, :])
    return np.concatenate(outs, axis=0), res


def kernel(inputs, params, bias):
    out, _ = run(inputs, params, bias)
    return out


# revision 53
# speedup vs baseline: 1.1503x; 1.1503x over previous
"""Trainium2 Bass kernel for the AR(t) recurrence problem.

Math: the recurrence  x_i = sum_j params[j] * x_{i-1-j} + bias  (i in [t, 2t))
is affine in the seed window:  out = inputs @ M + c,  with M, c functions of
params/bias only. M factors exactly as M = T2 @ (I + T1h) where T2 is the
lower-triangular Toeplitz of p_rev (= params reversed) and T1h the upper-
triangular Toeplitz of the AR impulse-response tail h~. The alternating-sign
params make T1h's error-carrying content coherent/smooth, so T1h is
numerically low-rank: a rank-128 SVD (host, randomized) captures it to
~4e-4 output error. Device compute per core (512 batch rows, fp8e4,
DoubleRow = 2 contraction k-tiles per PE pass):

  stage-1 (72 mm): psum1[ci] = f.T tiles       f.T = T2^T s.T  (triangular)
  stage-A ( 8 mm): psumY = (sA*T2@U)^T s.T  -> Y8               (rank-128)
  stage-B (16 mm): psum2[it] = (sB*V)^T Y8 + 2*f8[it]           (DoubleRow
                   pairs the V-tile with an exact 2*I fp8 tile, adding the
                   identity term in the same pass)
  psum2 = 2^11 (f + corr).T -> fp8e4 (Act/DVE alternating) -> DMA
  host: /2^11 + c

Schedule notes (from perfetto-trace iteration; measured ~40.1-41.8us vs
the 42.4-43.2us baseline, run-to-run chip power state adds ~15% noise):
  - Only sync+scalar have HWDGE queues (gpsimd dma_start is slow SWDGE).
  - HWDGE early-phase throughput is ~(50-130 rows/us) x row-bytes per
    queue, so input chunks use 1-3KB rows (host permutes DRAM pair order
    to [7,5,3,1 | 6,4,2,0]) and each queue's entries are ordered by when
    the warm MM stream consumes them; the first usable pair lands ~2.8us
    after the first trigger no matter what.
  - Warm-up matmuls on a zeroed tile keep the PE gaplessly busy from
    ~2.5us into the measured window; the HAM clock gate (1.2 -> 2.4 GHz)
    flips ~3.7us after the last PE idle gap, so the real stream runs at
    warm 216ns/pass almost from its start.
  - stage-A is interleaved right after the stage-1 groups that consume
    the same input pairs; stage-B pairs spread through the back half.
  - Output is fp8e4 at 2^11 scale (data term absmax ~0.115 -> 236 < 448
    saturation), halving output DMA bytes; out DMAs alternate sync and
    scalar queues and the final pair is cast in engine-split halves and
    sent as two single-tile DMAs to shorten the tail.

fp8 is ample precision: the data term has magnitude ~0.0155 rms while
|out| ~ 1.8 (dominated by c, exact). Measured max elementwise rel error
~5.2e-3 (tolerance 2e-2).
"""

import numpy as np
import ml_dtypes

B = 4096          # batch rows
T = 2048          # time steps == contraction length
NCORES = 8
BS = B // NCORES  # 512 rows per core
P = 128           # partitions
NJ = T // P       # 16 contraction tiles
NPAIR = NJ // 2   # 8 DoubleRow contraction pairs
RANK = 128        # low-rank correction rank
W_SCALE = 1024.0  # 2^10 on T2 tiles
SA = 128.0        # 2^7 on Pt (stage-A)
SB = 16.0         # 2^4 on V (stage-B); psum2 scale = W_SCALE * 2 = SA*SB = 2^11
OUT_SCALE = 2048.0
NWARM = 7

E4 = ml_dtypes.float8_e4m3

_cache = {}


def _build_and_compile():
    import concourse.mybir as mybir
    from concourse import bacc
    from concourse.tile import TileContext

    nc = bacc.Bacc(
        "TRN2",
        target_bir_lowering=False,
        debug=False,
        enable_asserts=False,
        num_devices=NCORES,
    )
    in8 = nc.dram_tensor("art_in8", [P, NJ, BS], mybir.dt.float8e4, kind="ExternalInput")
    w8 = nc.dram_tensor("art_w8", [P, NJ + 1, P], mybir.dt.float8e4, kind="ExternalInput")
    p8 = nc.dram_tensor("art_p8", [P, NJ, RANK], mybir.dt.float8e4, kind="ExternalInput")
    vi8 = nc.dram_tensor("art_vi8", [P, 18, P], mybir.dt.float8e4, kind="ExternalInput")
    # out.T in [partition, k-tile-pair-slot, b] layout so 2-tile SBUF groups DMA
    # with matching AP shapes; host reassembles.
    outT = nc.dram_tensor("art_outT", [P, NJ, BS], mybir.dt.float8e4, kind="ExternalOutput")

    DR = mybir.MatmulPerfMode.DoubleRow

    with TileContext(nc) as tc:
        with (
            tc.tile_pool(name="wstk", bufs=1) as wpool,
            tc.tile_pool(name="pstk", bufs=1) as ppool,
            tc.tile_pool(name="vstk", bufs=1) as vpool,
            tc.tile_pool(name="warm", bufs=1) as wupool,
            tc.tile_pool(name="acts", bufs=NPAIR) as ipool,
            tc.tile_pool(name="yf", bufs=1) as yfpool,
            tc.tile_pool(name="outs", bufs=8) as opool,
            tc.tile_pool(name="ps1", bufs=4, space="PSUM") as f1pool,
            tc.tile_pool(name="ps2", bufs=4, space="PSUM") as c2pool,
        ):
            # W stack split into two tiles (overlapping at slot 4) so the
            # first stage-1 matmuls gate on an 80KB DMA, not the full stack.
            wta = wpool.tile([P, 9, P], mybir.dt.float8e4, name="wa_sb")
            wtb = wpool.tile([P, 9, P], mybir.dt.float8e4, name="wb_sb")
            pt_ = ppool.tile([P, NJ, RANK], mybir.dt.float8e4, name="p_sb")
            vt_ = vpool.tile([P, 18, P], mybir.dt.float8e4, name="v_sb")
            # three Y/f stacks (Y8 duplicated) so stage-B waves gate only on
            # their own quarter's f-casts and can run mid-stage-1.
            yfA = yfpool.tile([P, 9, BS], mybir.dt.float8e4, name="yfA_sb")
            yfD = yfpool.tile([P, 2, BS], mybir.dt.float8e4, name="yfD_sb")
            wu = wupool.tile([P, 2, BS], mybir.dt.float8e4, name="wu_sb")
            nc.vector.memset(wu[:], 0.0)
            nc.vector.memset(yfD[:, 1, :], 0.0)

            # input tiles; DRAM pair order (host korder) is
            # [7, 5, 3, 1, 6, 4, 2, 0] so each queue's chunks are contiguous
            # with 1-3KB rows (HWDGE throughput ~ rows/us x row-bytes).
            t7 = ipool.tile([P, 2, BS], mybir.dt.float8e4, tag="in", name="in_t7")
            t53 = ipool.tile([P, 4, BS], mybir.dt.float8e4, tag="in", name="in_t53")
            t1 = ipool.tile([P, 2, BS], mybir.dt.float8e4, tag="in", name="in_t1")
            t6 = ipool.tile([P, 2, BS], mybir.dt.float8e4, tag="in", name="in_t6")
            t420 = ipool.tile([P, 6, BS], mybir.dt.float8e4, tag="in", name="in_t420")
            rhs_map = {
                7: (t7, 0), 5: (t53, 0), 3: (t53, 2), 1: (t1, 0),
                6: (t6, 0), 4: (t420, 0), 2: (t420, 2), 0: (t420, 4),
            }

            def in_rhs(r):
                tl, s = rhs_map[r]
                return tl[:, s : s + 2, :]

            # ladder ordered by when the warm MM stream consumes each chunk
            # (HWDGE drains each queue's entries in order at a limited rate);
            # pt is split so stage-A(7..4) gates on a 128KB half.
            nc.sync.dma_start(out=wta[:], in_=w8[:, 0:9, :])
            nc.scalar.dma_start(out=t6[:], in_=in8[:, 8:10, :])
            nc.sync.dma_start(out=t7[:], in_=in8[:, 0:2, :])
            nc.scalar.dma_start(out=pt_[:, 8:16, :], in_=p8[:, 8:16, :])
            nc.sync.dma_start(out=t53[:], in_=in8[:, 2:6, :])
            nc.scalar.dma_start(out=t420[:], in_=in8[:, 10:16, :])
            nc.sync.dma_start(out=wtb[:], in_=w8[:, 8:17, :])
            nc.scalar.dma_start(out=pt_[:, 0:8, :], in_=p8[:, 0:8, :])
            nc.sync.dma_start(out=t1[:], in_=in8[:, 6:8, :])
            nc.sync.dma_start(out=vt_[:], in_=vi8[:])

            # warm-up matmuls keep the PE gaplessly busy from early on so
            # the HAM clock (1.2 -> 2.4 GHz, flips ~3.7us after the last PE
            # idle gap) is warm soon after the real stream starts.
            wps = c2pool.tile([P, BS], mybir.dt.float32, tag="c", name="warmps")
            for _ in range(NWARM):
                nc.tensor.matmul(wps[:], wu[:, :, :P], wu[:], perf_mode=DR)

            psY = c2pool.tile([P, BS], mybir.dt.float32, tag="c", name="psY")

            def stage_a(r):
                nc.tensor.matmul(
                    psY[:],
                    pt_[:, 2 * r : 2 * r + 2, :],
                    in_rhs(r),
                    start=(r == NPAIR - 1),
                    stop=(r == 0),
                    perf_mode=DR,
                )
                if r == 0:
                    nc.vector.tensor_scalar_mul(yfA[:, 0, :], psY[:], 1.0)
                    nc.scalar.copy(yfD[:, 0, :], psY[:])

            def stage_1(ci, r):
                ps1 = f1pool.tile([P, BS], mybir.dt.float32, tag="f", name=f"f{ci}")
                inbank = ci <= 7
                for rp in range(r, NPAIR):
                    q = 2 * rp - ci + 1
                    lhsT = wta[:, q : q + 2, :] if q <= 7 else wtb[:, q - 8 : q - 6, :]
                    nc.tensor.matmul(
                        ps1[:],
                        lhsT,
                        in_rhs(rp),
                        start=(rp == r),
                        stop=(rp == NPAIR - 1) and not inbank,
                        perf_mode=DR,
                    )
                if inbank:
                    # fold the rank correction into the f psum (extra DR pass
                    # pairing the 2^3-scaled V tile with a zero slot), then
                    # cast psum -> fp8 out directly: no f8 round-trip, so the
                    # tail does not wait on an f-cast before stage-B.
                    nc.tensor.matmul(
                        ps1[:],
                        vt_[:, 16 - ci : 18 - ci, :],
                        yfD[:, 0:2, :],
                        start=False,
                        stop=True,
                        perf_mode=DR,
                    )
                    ot = opool.tile([P, 1, BS], mybir.dt.float8e4, tag="o", name=f"od{ci}")
                    h = BS // 2
                    nc.vector.tensor_scalar_mul(ot[:, 0, :h], ps1[:, :h], 1.0)
                    nc.scalar.copy(ot[:, 0, h:], ps1[:, h:])
                    q = nc.scalar if ci % 2 == 1 else nc.sync
                    q.dma_start(out=outT[:, ci : ci + 1, :], in_=ot[:])
                    return
                # evacuate f.T tile to fp8, alternating engines (ci >= 8)
                dst = yfA[:, ci - 7, :]
                if ci % 2 == 0:
                    nc.vector.tensor_scalar_mul(dst, ps1[:], 1.0)
                else:
                    nc.scalar.copy(dst, ps1[:])

            oq = {"i": 0}

            def stage_b_pair(hi, last=False):
                lo = hi - 1
                ot = opool.tile([P, 2, BS], mybir.dt.float8e4, tag="o", name=f"o{lo}")
                for it, slot in ((hi, 1), (lo, 0)):
                    ps2 = c2pool.tile([P, BS], mybir.dt.float32, tag="c", name=f"c{it}")
                    rhs = yfA[:, 0 : it - 6 : it - 7, :]
                    nc.tensor.matmul(
                        ps2[:],
                        vt_[:, it - 8 : 9 : 16 - it, :],
                        rhs,
                        start=True,
                        stop=True,
                        perf_mode=DR,
                    )
                    if last:
                        # split the final casts across both engines and DMA
                        # each tile separately on its own queue
                        h = BS // 2
                        nc.vector.tensor_scalar_mul(ot[:, slot, :h], ps2[:, :h], 1.0)
                        nc.scalar.copy(ot[:, slot, h:], ps2[:, h:])
                        q = nc.scalar if slot == 1 else nc.sync
                        q.dma_start(out=outT[:, lo + slot : lo + slot + 1, :],
                                    in_=ot[:, slot : slot + 1, :])
                    elif it % 2 == 1:
                        nc.scalar.copy(ot[:, slot, :], ps2[:])
                    else:
                        nc.vector.tensor_scalar_mul(ot[:, slot, :], ps2[:], 1.0)
                if not last:
                    q = nc.sync if oq["i"] % 2 == 0 else nc.scalar
                    oq["i"] += 1
                    q.dma_start(out=outT[:, lo : lo + 2, :], in_=ot[:])

            # interleave: first stage-1 tiles gate on the earliest DMAs, then
            # all stage-A passes (Y8 ready early), stage-1 descending, with
            # stage-B tiles 8-15 inserted as soon as yfA is complete.
            stage_1(15, 7)
            stage_1(14, 7)
            stage_1(13, 6)
            stage_1(12, 6)
            stage_a(7)
            stage_a(6)
            stage_1(11, 5)
            stage_1(10, 5)
            stage_a(5)
            stage_1(9, 4)
            stage_a(4)
            stage_1(8, 4)
            stage_a(3)
            stage_a(2)
            stage_a(1)
            stage_a(0)
            stage_1(7, 3)
            stage_1(6, 3)
            for hi in (15, 13, 11, 9):
                stage_b_pair(hi)
            stage_1(5, 2)
            stage_1(4, 2)
            stage_1(3, 1)
            stage_1(2, 1)
            stage_1(1, 0)
            stage_1(0, 0)

    nc.compile()
    return nc


def _host_factors(params, bias):
    """All device operand tensors + c, from params/bias (float64 host math)."""
    t = T
    p_rev = params[::-1].astype(np.float64)

    # c: bias propagation through the recurrence
    b = np.float64(bias[0])
    u = np.zeros(t, np.float64)
    c = np.empty(t, np.float64)
    for k in range(t):
        nv = u @ p_rev + b
        c[k] = nv
        u = np.roll(u, -1)
        u[-1] = nv

    # h~: AR impulse response tail (h_0 = 1 excluded)
    a_full = np.concatenate([[0.0], params.astype(np.float64)])
    h = np.zeros(t)
    h[0] = 1.0
    for dd in range(1, t):
        h[dd] = a_full[1:dd + 1] @ h[dd - 1::-1][:dd]
    ht = h.copy()
    ht[0] = 0.0

    idx = np.arange(t)
    D = idx[:, None] - idx[None, :]
    T2 = np.where(D >= 0, p_rev[np.clip(D, 0, t - 1)], 0.0)
    T1h = np.where(-D >= 1, ht[np.clip(-D, 0, t - 1)], 0.0)

    # randomized SVD of the correction operator
    rng = np.random.default_rng(0)
    Q, _ = np.linalg.qr(T1h @ rng.standard_normal((t, RANK + 32)))
    u2, sig, vt = np.linalg.svd(Q.T @ T1h, full_matrices=False)
    U = (Q @ u2[:, :RANK]) * sig[:RANK]
    Vt = vt[:RANK]
    Pt = T2 @ U  # [t, RANK]

    # W stack: slot s=0..16 <-> tile-diagonal d=s-1; W[s][jw,kw]=1024*p_rev[128(s-1)+jw-kw]
    sidx = (128 * (np.arange(NJ + 1) - 1))[:, None, None] + idx[:P, None] - idx[None, :P]
    wvals = np.where(
        (sidx >= 0) & (sidx < t), (W_SCALE * p_rev)[np.clip(sidx, 0, t - 1)], 0.0
    )  # [17, 128, 128]
    w8 = np.ascontiguousarray(wvals.transpose(1, 0, 2).astype(np.float32)).astype(E4)

    p8 = np.ascontiguousarray(
        (SA * Pt).reshape(NJ, P, RANK).transpose(1, 0, 2).astype(np.float32)
    ).astype(E4)

    vi = np.empty((P, 18, P), np.float32)
    # positions 0-7: V tiles 8..15 at SB (stage-B path); 8: the 2*I tile;
    # 9-16: V tiles 7..0 at W_SCALE/SA = 2^3 (in-bank correction passes,
    # psum stays at 2^10); 17: zeros (pairs with position 16).
    vi[:, 0:8, :] = (SB * Vt).reshape(P, NJ, P)[:, 8:16, :]
    vi[:, 8, :] = 2.0 * np.eye(P, dtype=np.float32)
    vb = (8.0 * Vt).reshape(P, NJ, P)
    for k in range(8):
        vi[:, 9 + k, :] = vb[:, 7 - k, :]
    vi[:, 17, :] = 0.0
    vi8 = np.ascontiguousarray(vi).astype(E4)

    return w8, p8, vi8, c


def _make_in_maps(inputs, params, bias):
    key = (params.tobytes(), bias.tobytes())
    if _cache.get("fkey") == key:
        w8, p8, vi8, c = _cache["factors"]
    else:
        w8, p8, vi8, c = _host_factors(params, bias)
        _cache["fkey"] = key
        _cache["factors"] = (w8, p8, vi8, c)
    in8_full = inputs.astype(E4)
    # DRAM pair order expected by the kernel's DMA ladder:
    # [7, 5, 3, 1, 6, 4, 2, 0]  (pair r = k-tiles 2r, 2r+1)
    korder = [14, 15, 10, 11, 6, 7, 2, 3, 12, 13, 8, 9, 4, 5, 0, 1]
    in_maps = []
    for s in range(NCORES):
        shard = in8_full[s * BS : (s + 1) * BS, :]  # [BS, T]
        in8 = np.ascontiguousarray(
            shard.T.reshape(NJ, P, BS)[korder].transpose(1, 0, 2)
        )
        in_maps.append({"art_in8": in8, "art_w8": w8, "art_p8": p8, "art_vi8": vi8})
    return in_maps, c


def run(inputs, params, bias, **spmd_kwargs):
    """Build in_maps, run the SPMD kernel, return (output, BassKernelResults)."""
    from concourse.bass_utils import run_bass_kernel_spmd

    if "nc" not in _cache:
        _cache["nc"] = _build_and_compile()
    nc = _cache["nc"]

    inputs = np.ascontiguousarray(np.asarray(inputs, dtype=np.float32))
    params = np.asarray(params, dtype=np.float32)
    bias = np.asarray(bias, dtype=np.float32)
    assert inputs.shape == (B, T), inputs.shape
    assert params.shape == (T,), params.shape
    in_maps, c = _make_in_maps(inputs, params, bias)
    res = run_bass_kernel_spmd(nc, in_maps, core_ids=list(range(NCORES)), **spmd_kwargs)
    slot_scale = np.full(NJ, 1.0 / OUT_SCALE, np.float32)
    slot_scale[:8] = 1.0 / W_SCALE  # in-bank tiles (0-7) at 2^10
    row_scale = np.repeat(slot_scale, P)  # out.T row = 128*slot + p
    c32 = c.astype(np.float32)
    outs = []
    for r in res.results:
        # art_outT [128, 16, 512]: [p, slot, b] with out.T row = 128*slot + p
        oT = r["art_outT"].astype(np.float32).transpose(1, 0, 2).reshape(T, BS)
        outs.append(oT.T * row_scale[None, :] + c32[None, :])
    return np.concatenate(outs, axis=0), res


def kernel(inputs, params, bias):
    out, _ = run(inputs, params, bias)
    return out


# revision 54
# speedup vs baseline: 1.1507x; 1.0004x over previous
"""Trainium2 Bass kernel for the AR(t) recurrence problem.

Math: the recurrence  x_i = sum_j params[j] * x_{i-1-j} + bias  (i in [t, 2t))
is affine in the seed window:  out = inputs @ M + c,  with M, c functions of
params/bias only. M factors exactly as M = T2 @ (I + T1h) where T2 is the
lower-triangular Toeplitz of p_rev (= params reversed) and T1h the upper-
triangular Toeplitz of the AR impulse-response tail h~. The alternating-sign
params make T1h's error-carrying content coherent/smooth, so T1h is
numerically low-rank: a rank-128 SVD (host, randomized) captures it to
~4e-4 output error. Device compute per core (512 batch rows, fp8e4,
DoubleRow = 2 contraction k-tiles per PE pass):

  stage-1 (72 mm): psum1[ci] = f.T tiles       f.T = T2^T s.T  (triangular)
  stage-A ( 8 mm): psumY = (sA*T2@U)^T s.T  -> Y8               (rank-128)
  stage-B (16 mm): psum2[it] = (sB*V)^T Y8 + 2*f8[it]           (DoubleRow
                   pairs the V-tile with an exact 2*I fp8 tile, adding the
                   identity term in the same pass)
  psum2 = 2^11 (f + corr).T -> fp8e4 (Act/DVE alternating) -> DMA
  host: /2^11 + c

Schedule notes (from perfetto-trace iteration; measured ~40.1-41.8us vs
the 42.4-43.2us baseline, run-to-run chip power state adds ~15% noise):
  - Only sync+scalar have HWDGE queues (gpsimd dma_start is slow SWDGE).
  - HWDGE early-phase throughput is ~(50-130 rows/us) x row-bytes per
    queue, so input chunks use 1-3KB rows (host permutes DRAM pair order
    to [7,5,3,1 | 6,4,2,0]) and each queue's entries are ordered by when
    the warm MM stream consumes them; the first usable pair lands ~2.8us
    after the first trigger no matter what.
  - Warm-up matmuls on a zeroed tile keep the PE gaplessly busy from
    ~2.5us into the measured window; the HAM clock gate (1.2 -> 2.4 GHz)
    flips ~3.7us after the last PE idle gap, so the real stream runs at
    warm 216ns/pass almost from its start.
  - stage-A is interleaved right after the stage-1 groups that consume
    the same input pairs; stage-B pairs spread through the back half.
  - Output is fp8e4 at 2^11 scale (data term absmax ~0.115 -> 236 < 448
    saturation), halving output DMA bytes; out DMAs alternate sync and
    scalar queues and the final pair is cast in engine-split halves and
    sent as two single-tile DMAs to shorten the tail.

fp8 is ample precision: the data term has magnitude ~0.0155 rms while
|out| ~ 1.8 (dominated by c, exact). Measured max elementwise rel error
~5.2e-3 (tolerance 2e-2).
"""

import numpy as np
import ml_dtypes

B = 4096          # batch rows
T = 2048          # time steps == contraction length
NCORES = 8
BS = B // NCORES  # 512 rows per core
P = 128           # partitions
NJ = T // P       # 16 contraction tiles
NPAIR = NJ // 2   # 8 DoubleRow contraction pairs
RANK = 128        # low-rank correction rank
W_SCALE = 1024.0  # 2^10 on T2 tiles
SA = 128.0        # 2^7 on Pt (stage-A)
SB = 16.0         # 2^4 on V (stage-B); psum2 scale = W_SCALE * 2 = SA*SB = 2^11
OUT_SCALE = 2048.0
NWARM = 7

E4 = ml_dtypes.float8_e4m3

_cache = {}


def _build_and_compile():
    import concourse.mybir as mybir
    from concourse import bacc
    from concourse.tile import TileContext

    nc = bacc.Bacc(
        "TRN2",
        target_bir_lowering=False,
        debug=False,
        enable_asserts=False,
        num_devices=NCORES,
    )
    in8 = nc.dram_tensor("art_in8", [P, NJ, BS], mybir.dt.float8e4, kind="ExternalInput")
    w8 = nc.dram_tensor("art_w8", [P, NJ + 1, P], mybir.dt.float8e4, kind="ExternalInput")
    p8 = nc.dram_tensor("art_p8", [P, NJ, RANK], mybir.dt.float8e4, kind="ExternalInput")
    vi8 = nc.dram_tensor("art_vi8", [P, 18, P], mybir.dt.float8e4, kind="ExternalInput")
    # out.T in [partition, k-tile-pair-slot, b] layout so 2-tile SBUF groups DMA
    # with matching AP shapes; host reassembles.
    outT = nc.dram_tensor("art_outT", [P, NJ, BS], mybir.dt.float8e4, kind="ExternalOutput")

    DR = mybir.MatmulPerfMode.DoubleRow

    with TileContext(nc) as tc:
        with (
            tc.tile_pool(name="wstk", bufs=1) as wpool,
            tc.tile_pool(name="pstk", bufs=1) as ppool,
            tc.tile_pool(name="vstk", bufs=1) as vpool,
            tc.tile_pool(name="warm", bufs=1) as wupool,
            tc.tile_pool(name="acts", bufs=NPAIR) as ipool,
            tc.tile_pool(name="yf", bufs=1) as yfpool,
            tc.tile_pool(name="outs", bufs=8) as opool,
            tc.tile_pool(name="ps1", bufs=4, space="PSUM") as f1pool,
            tc.tile_pool(name="ps2", bufs=4, space="PSUM") as c2pool,
        ):
            # W stack split into two tiles (overlapping at slot 4) so the
            # first stage-1 matmuls gate on an 80KB DMA, not the full stack.
            wta = wpool.tile([P, 9, P], mybir.dt.float8e4, name="wa_sb")
            wtb = wpool.tile([P, 9, P], mybir.dt.float8e4, name="wb_sb")
            pt_ = ppool.tile([P, NJ, RANK], mybir.dt.float8e4, name="p_sb")
            vt_ = vpool.tile([P, 18, P], mybir.dt.float8e4, name="v_sb")
            # three Y/f stacks (Y8 duplicated) so stage-B waves gate only on
            # their own quarter's f-casts and can run mid-stage-1.
            yfA = yfpool.tile([P, 9, BS], mybir.dt.float8e4, name="yfA_sb")
            yfD = yfpool.tile([P, 2, BS], mybir.dt.float8e4, name="yfD_sb")
            wu = wupool.tile([P, 2, BS], mybir.dt.float8e4, name="wu_sb")
            nc.vector.memset(wu[:], 0.0)
            nc.vector.memset(yfD[:, 1, :], 0.0)

            # input tiles; DRAM pair order (host korder) is
            # [7, 5, 3, 1, 6, 4, 2, 0] so each queue's chunks are contiguous
            # with 1-3KB rows (HWDGE throughput ~ rows/us x row-bytes).
            t7 = ipool.tile([P, 2, BS], mybir.dt.float8e4, tag="in", name="in_t7")
            t53 = ipool.tile([P, 4, BS], mybir.dt.float8e4, tag="in", name="in_t53")
            t1 = ipool.tile([P, 2, BS], mybir.dt.float8e4, tag="in", name="in_t1")
            t6 = ipool.tile([P, 2, BS], mybir.dt.float8e4, tag="in", name="in_t6")
            t4 = ipool.tile([P, 2, BS], mybir.dt.float8e4, tag="in", name="in_t4")
            t20 = ipool.tile([P, 4, BS], mybir.dt.float8e4, tag="in", name="in_t20")
            rhs_map = {
                7: (t7, 0), 5: (t53, 0), 3: (t53, 2), 1: (t1, 0),
                6: (t6, 0), 4: (t4, 0), 2: (t20, 0), 0: (t20, 2),
            }

            def in_rhs(r):
                tl, s = rhs_map[r]
                return tl[:, s : s + 2, :]

            # ladder ordered by when the warm MM stream consumes each chunk
            # (HWDGE drains each queue's entries in order at a limited rate);
            # pt is split so stage-A(7..4) gates on a 128KB half.
            nc.sync.dma_start(out=wta[:], in_=w8[:, 0:9, :])
            nc.scalar.dma_start(out=t6[:], in_=in8[:, 8:10, :])
            nc.sync.dma_start(out=t7[:], in_=in8[:, 0:2, :])
            nc.scalar.dma_start(out=pt_[:, 8:16, :], in_=p8[:, 8:16, :])
            nc.sync.dma_start(out=t53[:], in_=in8[:, 2:6, :])
            nc.scalar.dma_start(out=t4[:], in_=in8[:, 10:12, :])
            nc.scalar.dma_start(out=t20[:], in_=in8[:, 12:16, :])
            nc.sync.dma_start(out=wtb[:], in_=w8[:, 8:17, :])
            nc.scalar.dma_start(out=pt_[:, 0:8, :], in_=p8[:, 0:8, :])
            nc.sync.dma_start(out=t1[:], in_=in8[:, 6:8, :])
            nc.sync.dma_start(out=vt_[:], in_=vi8[:])

            # warm-up matmuls keep the PE gaplessly busy from early on so
            # the HAM clock (1.2 -> 2.4 GHz, flips ~3.7us after the last PE
            # idle gap) is warm soon after the real stream starts.
            wps = c2pool.tile([P, BS], mybir.dt.float32, tag="c", name="warmps")
            for _ in range(NWARM):
                nc.tensor.matmul(wps[:], wu[:, :, :P], wu[:], perf_mode=DR)

            psY = c2pool.tile([P, BS], mybir.dt.float32, tag="c", name="psY")

            def stage_a(r):
                nc.tensor.matmul(
                    psY[:],
                    pt_[:, 2 * r : 2 * r + 2, :],
                    in_rhs(r),
                    start=(r == NPAIR - 1),
                    stop=(r == 0),
                    perf_mode=DR,
                )
                if r == 0:
                    nc.vector.tensor_scalar_mul(yfA[:, 0, :], psY[:], 1.0)
                    nc.scalar.copy(yfD[:, 0, :], psY[:])

            def stage_1(ci, r):
                ps1 = f1pool.tile([P, BS], mybir.dt.float32, tag="f", name=f"f{ci}")
                inbank = ci <= 7
                for rp in range(r, NPAIR):
                    q = 2 * rp - ci + 1
                    lhsT = wta[:, q : q + 2, :] if q <= 7 else wtb[:, q - 8 : q - 6, :]
                    nc.tensor.matmul(
                        ps1[:],
                        lhsT,
                        in_rhs(rp),
                        start=(rp == r),
                        stop=(rp == NPAIR - 1) and not inbank,
                        perf_mode=DR,
                    )
                if inbank:
                    # fold the rank correction into the f psum (extra DR pass
                    # pairing the 2^3-scaled V tile with a zero slot), then
                    # cast psum -> fp8 out directly: no f8 round-trip, so the
                    # tail does not wait on an f-cast before stage-B.
                    nc.tensor.matmul(
                        ps1[:],
                        vt_[:, 16 - ci : 18 - ci, :],
                        yfD[:, 0:2, :],
                        start=False,
                        stop=True,
                        perf_mode=DR,
                    )
                    ot = opool.tile([P, 1, BS], mybir.dt.float8e4, tag="o", name=f"od{ci}")
                    h = BS // 2
                    nc.vector.tensor_scalar_mul(ot[:, 0, :h], ps1[:, :h], 1.0)
                    nc.scalar.copy(ot[:, 0, h:], ps1[:, h:])
                    q = nc.scalar if ci % 2 == 1 else nc.sync
                    q.dma_start(out=outT[:, ci : ci + 1, :], in_=ot[:])
                    return
                # evacuate f.T tile to fp8, alternating engines (ci >= 8)
                dst = yfA[:, ci - 7, :]
                if ci % 2 == 0:
                    nc.vector.tensor_scalar_mul(dst, ps1[:], 1.0)
                else:
                    nc.scalar.copy(dst, ps1[:])

            oq = {"i": 0}

            def stage_b_pair(hi, last=False):
                lo = hi - 1
                ot = opool.tile([P, 2, BS], mybir.dt.float8e4, tag="o", name=f"o{lo}")
                for it, slot in ((hi, 1), (lo, 0)):
                    ps2 = c2pool.tile([P, BS], mybir.dt.float32, tag="c", name=f"c{it}")
                    rhs = yfA[:, 0 : it - 6 : it - 7, :]
                    nc.tensor.matmul(
                        ps2[:],
                        vt_[:, it - 8 : 9 : 16 - it, :],
                        rhs,
                        start=True,
                        stop=True,
                        perf_mode=DR,
                    )
                    if last:
                        # split the final casts across both engines and DMA
                        # each tile separately on its own queue
                        h = BS // 2
                        nc.vector.tensor_scalar_mul(ot[:, slot, :h], ps2[:, :h], 1.0)
                        nc.scalar.copy(ot[:, slot, h:], ps2[:, h:])
                        q = nc.scalar if slot == 1 else nc.sync
                        q.dma_start(out=outT[:, lo + slot : lo + slot + 1, :],
                                    in_=ot[:, slot : slot + 1, :])
                    elif it % 2 == 1:
                        nc.scalar.copy(ot[:, slot, :], ps2[:])
                    else:
                        nc.vector.tensor_scalar_mul(ot[:, slot, :], ps2[:], 1.0)
                if not last:
                    q = nc.sync if oq["i"] % 2 == 0 else nc.scalar
                    oq["i"] += 1
                    q.dma_start(out=outT[:, lo : lo + 2, :], in_=ot[:])

            # interleave: first stage-1 tiles gate on the earliest DMAs, then
            # all stage-A passes (Y8 ready early), stage-1 descending, with
            # stage-B tiles 8-15 inserted as soon as yfA is complete.
            stage_1(15, 7)
            stage_1(14, 7)
            stage_1(13, 6)
            stage_1(12, 6)
            stage_a(7)
            stage_a(6)
            stage_1(11, 5)
            stage_1(10, 5)
            stage_a(5)
            stage_1(9, 4)
            stage_a(4)
            stage_1(8, 4)
            stage_a(3)
            stage_a(2)
            stage_a(1)
            stage_a(0)
            stage_1(7, 3)
            stage_1(6, 3)
            for hi in (15, 13, 11, 9):
                stage_b_pair(hi)
            stage_1(5, 2)
            stage_1(4, 2)
            stage_1(3, 1)
            stage_1(2, 1)
            stage_1(1, 0)
            stage_1(0, 0)

    nc.compile()
    return nc


def _host_factors(params, bias):
    """All device operand tensors + c, from params/bias (float64 host math)."""
    t = T
    p_rev = params[::-1].astype(np.float64)

    # c: bias propagation through the recurrence
    b = np.float64(bias[0])
    u = np.zeros(t, np.float64)
    c = np.empty(t, np.float64)
    for k in range(t):
        nv = u @ p_rev + b
        c[k] = nv
        u = np.roll(u, -1)
        u[-1] = nv

    # h~: AR impulse response tail (h_0 = 1 excluded)
    a_full = np.concatenate([[0.0], params.astype(np.float64)])
    h = np.zeros(t)
    h[0] = 1.0
    for dd in range(1, t):
        h[dd] = a_full[1:dd + 1] @ h[dd - 1::-1][:dd]
    ht = h.copy()
    ht[0] = 0.0

    idx = np.arange(t)
    D = idx[:, None] - idx[None, :]
    T2 = np.where(D >= 0, p_rev[np.clip(D, 0, t - 1)], 0.0)
    T1h = np.where(-D >= 1, ht[np.clip(-D, 0, t - 1)], 0.0)

    # randomized SVD of the correction operator
    rng = np.random.default_rng(0)
    Q, _ = np.linalg.qr(T1h @ rng.standard_normal((t, RANK + 32)))
    u2, sig, vt = np.linalg.svd(Q.T @ T1h, full_matrices=False)
    U = (Q @ u2[:, :RANK]) * sig[:RANK]
    Vt = vt[:RANK]
    Pt = T2 @ U  # [t, RANK]

    # W stack: slot s=0..16 <-> tile-diagonal d=s-1; W[s][jw,kw]=1024*p_rev[128(s-1)+jw-kw]
    sidx = (128 * (np.arange(NJ + 1) - 1))[:, None, None] + idx[:P, None] - idx[None, :P]
    wvals = np.where(
        (sidx >= 0) & (sidx < t), (W_SCALE * p_rev)[np.clip(sidx, 0, t - 1)], 0.0
    )  # [17, 128, 128]
    w8 = np.ascontiguousarray(wvals.transpose(1, 0, 2).astype(np.float32)).astype(E4)

    p8 = np.ascontiguousarray(
        (SA * Pt).reshape(NJ, P, RANK).transpose(1, 0, 2).astype(np.float32)
    ).astype(E4)

    vi = np.empty((P, 18, P), np.float32)
    # positions 0-7: V tiles 8..15 at SB (stage-B path); 8: the 2*I tile;
    # 9-16: V tiles 7..0 at W_SCALE/SA = 2^3 (in-bank correction passes,
    # psum stays at 2^10); 17: zeros (pairs with position 16).
    vi[:, 0:8, :] = (SB * Vt).reshape(P, NJ, P)[:, 8:16, :]
    vi[:, 8, :] = 2.0 * np.eye(P, dtype=np.float32)
    vb = (8.0 * Vt).reshape(P, NJ, P)
    for k in range(8):
        vi[:, 9 + k, :] = vb[:, 7 - k, :]
    vi[:, 17, :] = 0.0
    vi8 = np.ascontiguousarray(vi).astype(E4)

    return w8, p8, vi8, c


def _make_in_maps(inputs, params, bias):
    key = (params.tobytes(), bias.tobytes())
    if _cache.get("fkey") == key:
        w8, p8, vi8, c = _cache["factors"]
    else:
        w8, p8, vi8, c = _host_factors(params, bias)
        _cache["fkey"] = key
        _cache["factors"] = (w8, p8, vi8, c)
    in8_full = inputs.astype(E4)
    # DRAM pair order expected by the kernel's DMA ladder:
    # [7, 5, 3, 1, 6, 4, 2, 0]  (pair r = k-tiles 2r, 2r+1)
    korder = [14, 15, 10, 11, 6, 7, 2, 3, 12, 13, 8, 9, 4, 5, 0, 1]
    in_maps = []
    for s in range(NCORES):
        shard = in8_full[s * BS : (s + 1) * BS, :]  # [BS, T]
        in8 = np.ascontiguousarray(
            shard.T.reshape(NJ, P, BS)[korder].transpose(1, 0, 2)
        )
        in_maps.append({"art_in8": in8, "art_w8": w8, "art_p8": p8, "art_vi8": vi8})
    return in_maps, c


def run(inputs, params, bias, **spmd_kwargs):
    """Build in_maps, run the SPMD kernel, return (output, BassKernelResults)."""
    from concourse.bass_utils import run_bass_kernel_spmd

    if "nc" not in _cache:
        _cache["nc"] = _build_and_compile()
    nc = _cache["nc"]

    inputs = np.ascontiguousarray(np.asarray(inputs, dtype=np.float32))
    params = np.asarray(params, dtype=np.float32)
    bias = np.asarray(bias, dtype=np.float32)
    assert inputs.shape == (B, T), inputs.shape
    assert params.shape == (T,), params.shape
    in_maps, c = _make_in_maps(inputs, params, bias)
    res = run_bass_kernel_spmd(nc, in_maps, core_ids=list(range(NCORES)), **spmd_kwargs)
    slot_scale = np.full(NJ, 1.0 / OUT_SCALE, np.float32)
    slot_scale[:8] = 1.0 / W_SCALE  # in-bank tiles (0-7) at 2^10
    row_scale = np.repeat(slot_scale, P)  # out.T row = 128*slot + p
    c32 = c.astype(np.float32)
    outs = []
    for r in res.results:
        # art_outT [128, 16, 512]: [p, slot, b] with out.T row = 128*slot + p
        oT = r["art_outT"].astype(np.float32).transpose(1, 0, 2).reshape(T, BS)
        outs.append(oT.T * row_scale[None, :] + c32[None, :])
    return np.concatenate(outs, axis=0), res


def kernel(inputs, params, bias):
    out, _ = run(inputs, params, bias)
    return out
